# revision 6
# baseline (speedup 1.0000x reference)
"""Trainium2 Bass kernel for nn_MeshLoss (chamfer-to-top-surface + fem MSE).

Computation (see reference):
  top  = network_mesh[:, :, :, -1, :]    -> B x 1024 "top surface" points (3D)
  dist2[b, m] = min_n || pc[b,:,m] - top[b,:,n] ||^2
  out = mean(dist2) + mean((network_mesh[...,:15,:] - fem_mesh[...,:15,:])**2)

Distribution: 8 cores = (B=4) x (M-half=2). Each core computes a partial
scalar; host sums the 8x3 partials.

Per-core algorithm (all fp32, exact):
  Pair trick: min(da, db) = (da+db)/2 - |da-db|/2, where da = ||p-ta||^2 - ||p||^2.
  Two K=4 matmuls per 128-point tile produce s' = (da+db)/2 and d' = (da-db)/2
  for 512 top-pairs; ACT computes |d'| (PSUM->SBUF), DVE does a fused
  subtract+min-reduce (tensor_tensor_reduce) -> per-point min. ||p||^2 and the
  fem MSE are ACT square+accumulate passes. Final partition reduction is a
  ones-vector matmul; host adds the 3 partial sums per core.
"""

import os
import numpy as np
from contextlib import ExitStack

B = 4
M = 16384
MSHARD = M // 2          # 8192 points per core
N = 1024                 # top surface points per batch
NPAIR = N // 2           # 512
MT = MSHARD // 128       # 64 m-tiles per core
FEM_ELEMS = 3 * 16 * 15 * 32    # per-core fem slice elements = 23040 = 128*180
CHAMFER_SCALE = 1.0 / float(B * M)          # 1/65536
FEM_SCALE = 1.0 / float(B * 3 * 32 * 15 * 32)   # 1/184320
WEIGHT = 1.0

_NC_CACHE = {}


def _build_nc():
    import concourse.bacc as bacc
    import concourse.tile as tile
    import concourse.mybir as mybir

    f32 = mybir.dt.float32
    ACTF = mybir.ActivationFunctionType
    ALU = mybir.AluOpType

    nc = bacc.Bacc("TRN2", target_bir_lowering=False, debug=False, num_devices=8)

    tops_d = nc.dram_tensor("tops", [3, N], f32, kind="ExternalInput").ap()
    pcs4_d = nc.dram_tensor("pcs4", [4, MSHARD], f32, kind="ExternalInput").ap()
    pcsf_d = nc.dram_tensor("pcsf", [128, 192], f32, kind="ExternalInput").ap()
    nmb_d = nc.dram_tensor("nmb", [128, 180], f32, kind="ExternalInput").ap()
    femb_d = nc.dram_tensor("femb", [128, 180], f32, kind="ExternalInput").ap()
    ones_d = nc.dram_tensor("ones", [128, 1], f32, kind="ExternalInput").ap()
    out_d = nc.dram_tensor("out", [1, 3], f32, kind="ExternalOutput").ap()

    with tile.TileContext(nc) as tc, ExitStack() as ctx:
        const = ctx.enter_context(tc.tile_pool(name="const", bufs=1))
        sb = ctx.enter_context(tc.tile_pool(name="sb", bufs=3))
        junk = ctx.enter_context(tc.tile_pool(name="junk", bufs=2))
        ps_pool = ctx.enter_context(tc.tile_pool(name="psums", bufs=3, space="PSUM"))
        pd_pool = ctx.enter_context(tc.tile_pool(name="psumd", bufs=3, space="PSUM"))
        pt_pool = ctx.enter_context(tc.tile_pool(name="psumt", bufs=1, space="PSUM"))

        # ---------- loads ----------
        top_sb = const.tile([3, N], f32, tag="top")
        nc.sync.dma_start(top_sb[:], tops_d[:])
        pcs_sb = const.tile([4, MSHARD], f32, tag="pcs")
        nc.sync.dma_start(pcs_sb[:], pcs4_d[:])
        ones_sb = const.tile([128, 1], f32, tag="ones")
        nc.sync.dma_start(ones_sb[:], ones_d[:])
        pcsf_sb = const.tile([128, 192], f32, tag="pcsf")
        nc.sync.dma_start(pcsf_sb[:], pcsf_d[:])
        nmb_sb = const.tile([128, 180], f32, tag="nmb")
        nc.sync.dma_start(nmb_sb[:], nmb_d[:])
        femb_sb = const.tile([128, 180], f32, tag="femb")
        nc.sync.dma_start(femb_sb[:], femb_d[:])

        # ---------- prep: t4 = [-2t ; ||t||^2] ----------
        # dot(p4, t4[:, n]) = ||p - t_n||^2 - ||p||^2  (||p||^2 added via cols)
        sqt = const.tile([3, N], f32, tag="sqt")
        nc.scalar.activation(sqt[:], top_sb[:], ACTF.Square)
        pt = pt_pool.tile([1, N], f32, tag="pt")
        nc.tensor.matmul(pt[:, 0:512], ones_sb[0:3, :], sqt[:, 0:512],
                         start=True, stop=True)
        nc.tensor.matmul(pt[:, 512:1024], ones_sb[0:3, :], sqt[:, 512:1024],
                         start=True, stop=True)
        t4 = const.tile([4, N], f32, tag="t4")
        nc.scalar.activation(t4[0:3, :], top_sb[:], ACTF.Copy, scale=-2.0)
        # engines can't write at partition offset 3; stage on partition 0, DMA in
        normsq = const.tile([1, N], f32, tag="normsq")
        nc.scalar.activation(normsq[:], pt[:], ACTF.Copy)
        nc.sync.dma_start(t4[3:4, :], normsq[:])

        mins = const.tile([128, MT], f32, tag="mins")
        cols = const.tile([128, 3], f32, tag="cols")

        # ---------- ||p||^2 and fem MSE partials ----------
        p2j = junk.tile([128, 192], f32, tag="p2j")
        nc.scalar.activation(p2j[:], pcsf_sb[:], ACTF.Square,
                             scale=float(np.sqrt(CHAMFER_SCALE)),
                             accum_out=cols[:, 1:2])
        fdiff = junk.tile([128, 180], f32, tag="fdiff")
        nc.vector.tensor_sub(fdiff[:], nmb_sb[:], femb_sb[:])
        fj = junk.tile([128, 180], f32, tag="fj")
        nc.scalar.activation(fj[:], fdiff[:], ACTF.Square,
                             scale=float(np.sqrt(FEM_SCALE * WEIGHT)),
                             accum_out=cols[:, 2:3])

        # ---------- main chamfer loop ----------
        # bankA = dots vs tops 0..511, bankB = dots vs tops 512..1023
        # ACT copies bankB to SBUF; DVE: pm = min(bankA, bs); mins = min(pm)*scale
        for mt in range(MT):
            lhsT = pcs_sb[:, mt * 128:(mt + 1) * 128]
            ps = ps_pool.tile([128, NPAIR], f32, tag="ps")
            nc.tensor.matmul(ps[:], lhsT, t4[:, 0:NPAIR], start=True, stop=True)
            pd = pd_pool.tile([128, NPAIR], f32, tag="pd")
            nc.tensor.matmul(pd[:], lhsT, t4[:, NPAIR:N], start=True, stop=True)
            bs = sb.tile([128, NPAIR], f32, tag="bs")
            nc.scalar.activation(bs[:], pd[:], ACTF.Copy)
            pm = junk.tile([128, NPAIR], f32, tag="pm")
            nc.vector.tensor_tensor(pm[:], ps[:], bs[:], op=ALU.min)
            tsj = junk.tile([128, NPAIR], f32, tag="tsj")
            nc.vector.tensor_scalar(tsj[:], pm[:], CHAMFER_SCALE, None,
                                    ALU.mult, ALU.min,
                                    accum_out=mins[:, mt:mt + 1])

        # ---------- final reduction ----------
        nc.vector.reduce_sum(cols[:, 0:1], mins[:], axis=mybir.AxisListType.X)
        pf = pt_pool.tile([1, 3], f32, tag="pt")
        nc.tensor.matmul(pf[:], ones_sb[:], cols[:], start=True, stop=True)
        out_sb = const.tile([1, 3], f32, tag="outsb")
        nc.scalar.activation(out_sb[:], pf[:], ACTF.Copy)
        nc.sync.dma_start(out_d[:], out_sb[:])

    nc.compile()
    return nc


def get_nc():
    if "nc" not in _NC_CACHE:
        _NC_CACHE["nc"] = _build_nc()
    return _NC_CACHE["nc"]


def shard_inputs(network_mesh, pc, fem_mesh):
    """Build the 8 per-core input maps (numpy slicing/layout only)."""
    network_mesh = np.ascontiguousarray(np.asarray(network_mesh, dtype=np.float32))
    pc = np.ascontiguousarray(np.asarray(pc, dtype=np.float32))
    fem_mesh = np.ascontiguousarray(np.asarray(fem_mesh, dtype=np.float32))
    ones_col = np.ones((128, 1), dtype=np.float32)
    in_maps = []
    for k in range(8):
        b, h = k // 2, k % 2
        tops = np.ascontiguousarray(network_mesh[b, :, :, 15, :].reshape(3, N))
        pcs = pc[b, :, h * MSHARD:(h + 1) * MSHARD]
        pcs4 = np.concatenate([pcs, np.ones((1, MSHARD), np.float32)], axis=0)
        pcs4 = np.ascontiguousarray(pcs4)
        pcsf = np.ascontiguousarray(pcs.reshape(128, 192))
        nmb = np.ascontiguousarray(
            network_mesh[b, :, h * 16:(h + 1) * 16, 0:15, :].reshape(128, 180))
        femb = np.ascontiguousarray(
            fem_mesh[b, :, h * 16:(h + 1) * 16, 0:15, :].reshape(128, 180))
        in_maps.append({
            "tops": tops, "pcs4": pcs4, "pcsf": pcsf,
            "nmb": nmb, "femb": femb, "ones": ones_col,
        })
    return in_maps


def kernel(network_mesh, pc, fem_mesh):
    from concourse.bass_utils import run_bass_kernel_spmd

    nc = get_nc()
    in_maps = shard_inputs(network_mesh, pc, fem_mesh)
    res = run_bass_kernel_spmd(nc, in_maps, list(range(8)))
    total = np.float64(0.0)
    for r in res.results:
        total += np.float64(np.sum(np.asarray(r["out"], dtype=np.float64)))
    return np.float32(total)


# revision 8
# speedup vs baseline: 1.8950x; 1.8950x over previous
"""Trainium2 Bass kernel for nn_MeshLoss (chamfer-to-top-surface + fem MSE).

Computation (see reference):
  top  = network_mesh[:, :, :, -1, :]    -> B x 1024 "top surface" points (3D)
  dist2[b, m] = min_n || pc[b,:,m] - top[b,:,n] ||^2
  out = mean(dist2) + mean((network_mesh[...,:15,:] - fem_mesh[...,:15,:])**2)

Distribution: 8 cores = (B=4) x (M-half=2). Each core computes a partial
scalar; host sums the 8x3 partials.

Per-core algorithm:
  dot(p~, t~_n) = ||p - t_n||^2 - ||p||^2 with p~ = [p;1], t~ = [-2t; ||t||^2].
  Matmuls run in bf16 hi/lo form stacked to K=16 (hh+hl+lh+ll accumulated in
  fp32 PSUM -> ~fp32-accurate dots, single-pass bf16-speed matmuls).
  Per 128-point tile: bankA = dots vs tops 0:512, bankB = vs tops 512:1024.
  ACT copies bankB to SBUF, DVE tensor_tensor-min(A, Bcopy) -> pm (batch-2
  tiles per op), then one 3D tensor_reduce-min per 8 tiles -> per-point mins.
  ||p||^2 and fem MSE are ACT square+accumulate passes. Final partition
  reduction is a ones-vector matmul; host adds the 3 partials per core.
"""

import numpy as np
import ml_dtypes
from contextlib import ExitStack

B = 4
M = 16384
MSHARD = M // 2          # 8192 points per core
N = 1024                 # top surface points per batch
NH = N // 2              # 512 = bank width
MT = MSHARD // 128       # 64 m-tiles per core
CHAMFER_SCALE = 1.0 / float(B * M)          # 1/65536
FEM_SCALE = 1.0 / float(B * 3 * 32 * 15 * 32)   # 1/184320
WEIGHT = 1.0
TTB = 2                  # m-tiles per TT-min op (PSUM batch)
RDB = 8                  # m-tiles per 3D-reduce op

_NC_CACHE = {}


def _build_nc():
    import concourse.bacc as bacc
    import concourse.tile as tile
    import concourse.mybir as mybir

    f32 = mybir.dt.float32
    bf16 = mybir.dt.bfloat16
    ACTF = mybir.ActivationFunctionType
    ALU = mybir.AluOpType

    nc = bacc.Bacc("TRN2", target_bir_lowering=False, debug=False, num_devices=8)

    tops_d = nc.dram_tensor("tops", [3, N], f32, kind="ExternalInput").ap()
    pcsf_d = nc.dram_tensor("pcsf", [96, 256], f32, kind="ExternalInput").ap()
    nmb_d = nc.dram_tensor("nmb", [128, 180], f32, kind="ExternalInput").ap()
    femb_d = nc.dram_tensor("femb", [128, 180], f32, kind="ExternalInput").ap()
    ones_d = nc.dram_tensor("ones", [128, 1], f32, kind="ExternalInput").ap()
    ozrow_d = nc.dram_tensor("ozrow", [2, MSHARD], bf16, kind="ExternalInput").ap()
    out_d = nc.dram_tensor("out", [1, 3], f32, kind="ExternalOutput").ap()

    with tile.TileContext(nc) as tc, ExitStack() as ctx:
        const = ctx.enter_context(tc.tile_pool(name="const", bufs=1))
        sb = ctx.enter_context(tc.tile_pool(name="sb", bufs=3))
        pmpool = ctx.enter_context(tc.tile_pool(name="pmp", bufs=2))
        psum = ctx.enter_context(tc.tile_pool(name="psum", bufs=2, space="PSUM"))

        # ---------- loads ----------
        top_sb = const.tile([3, N], f32, tag="top")
        nc.sync.dma_start(top_sb[:], tops_d[:])
        ones_sb = const.tile([128, 1], f32, tag="ones")
        nc.sync.dma_start(ones_sb[:], ones_d[:])
        pcsf_sb = const.tile([96, 256], f32, tag="pcsf")
        nc.sync.dma_start(pcsf_sb[:], pcsf_d[:])
        nmb_sb = const.tile([128, 180], f32, tag="nmb")
        nc.sync.dma_start(nmb_sb[:], nmb_d[:])
        femb_sb = const.tile([128, 180], f32, tag="femb")
        nc.sync.dma_start(femb_sb[:], femb_d[:])

        # ---------- prep: t4 = [-2t ; ||t||^2] (fp32) ----------
        sqt = const.tile([3, N], f32, tag="sqt")
        nc.scalar.activation(sqt[:], top_sb[:], ACTF.Square)
        pt = psum.tile([1, N], f32, tag="ps")
        nc.tensor.matmul(pt[:, 0:NH], ones_sb[0:3, :], sqt[:, 0:NH],
                         start=True, stop=True)
        nc.tensor.matmul(pt[:, NH:N], ones_sb[0:3, :], sqt[:, NH:N],
                         start=True, stop=True)
        t4 = const.tile([4, N], f32, tag="t4")
        nc.scalar.activation(t4[0:3, :], top_sb[:], ACTF.Copy, scale=-2.0)
        normsq = const.tile([1, N], f32, tag="normsq")
        nc.scalar.activation(normsq[:], pt[:], ACTF.Copy)
        nc.sync.dma_start(t4[3:4, :], normsq[:])

        # ---------- bf16 hi/lo decomposition ----------
        # rhs16 = [t4_hi; t4_lo; t4_hi; t4_lo]  (K=16)
        th = const.tile([4, N], bf16, tag="th")
        nc.vector.tensor_copy(th[:], t4[:])
        tl = const.tile([4, N], bf16, tag="tl")
        nc.vector.tensor_sub(tl[:], t4[:], th[:])
        t16 = const.tile([16, N], bf16, tag="t16")
        nc.sync.dma_start(t16[0:4, :], th[:])
        nc.sync.dma_start(t16[4:8, :], tl[:])
        nc.sync.dma_start(t16[8:12, :], th[:])
        nc.sync.dma_start(t16[12:16, :], tl[:])

        # lhsT16 = [p_hi;1; p_hi;1; p_lo;0; p_lo;0]  (K=16, M=8192)
        # p hi/lo computed on the [128,192] layout, DMA'd into the [3,8192]
        # rows (identical flat element order).
        ph = const.tile([96, 256], bf16, tag="ph")
        nc.vector.tensor_copy(ph[:], pcsf_sb[:])
        pl = const.tile([96, 256], bf16, tag="pl")
        nc.vector.tensor_sub(pl[:], pcsf_sb[:], ph[:])
        p16 = const.tile([16, MSHARD], bf16, tag="p16")
        nc.sync.dma_start(p16[0:3, :], ph[:])
        nc.sync.dma_start(p16[3:4, :], ozrow_d[0:1, :])
        nc.sync.dma_start(p16[4:7, :], ph[:])
        nc.sync.dma_start(p16[7:8, :], ozrow_d[0:1, :])
        nc.sync.dma_start(p16[8:11, :], pl[:])
        nc.sync.dma_start(p16[11:12, :], ozrow_d[1:2, :])
        nc.sync.dma_start(p16[12:15, :], pl[:])
        nc.sync.dma_start(p16[15:16, :], ozrow_d[1:2, :])

        mins = const.tile([128, MT], f32, tag="mins")
        cols = const.tile([128, 3], f32, tag="cols")
        nc.vector.memset(cols[:], 0.0)

        # ---------- ||p||^2 and fem MSE partials ----------
        p2j = pmpool.tile([96, 256], f32, tag="p2j")
        nc.scalar.activation(p2j[:], pcsf_sb[:], ACTF.Square,
                             scale=float(np.sqrt(CHAMFER_SCALE)),
                             accum_out=cols[0:96, 1:2])
        fdiff = pmpool.tile([128, 180], f32, tag="fdiff")
        nc.vector.tensor_sub(fdiff[:], nmb_sb[:], femb_sb[:])
        fj = pmpool.tile([128, 180], f32, tag="fj")
        nc.scalar.activation(fj[:], fdiff[:], ACTF.Square,
                             scale=float(np.sqrt(FEM_SCALE * WEIGHT)),
                             accum_out=cols[:, 2:3])

        # ---------- main chamfer loop ----------
        # PSUM slot [128, 2048] = [A1|B1|A2|B2] for 2 m-tiles.
        assert MT % RDB == 0 and RDB % TTB == 0
        for grp in range(MT // RDB):       # 8 groups of 8 m-tiles
            pmbig = pmpool.tile([128, RDB * NH], f32, tag="pmbig")
            pm3 = pmbig[:].rearrange("p (g n) -> p g n", g=RDB)
            for sub in range(RDB // TTB):  # 4 PSUM slots of 2 m-tiles
                ps = psum.tile([128, TTB * N], f32, tag="ps")
                for j in range(TTB):
                    mt = grp * RDB + sub * TTB + j
                    lhsT = p16[:, mt * 128:(mt + 1) * 128]
                    nc.tensor.matmul(ps[:, j * N:j * N + NH],
                                     lhsT, t16[:, 0:NH], start=True, stop=True)
                    nc.tensor.matmul(ps[:, j * N + NH:(j + 1) * N],
                                     lhsT, t16[:, NH:N], start=True, stop=True)
                ps3 = ps[:].rearrange("p (g n) -> p g n", g=2 * TTB)
                bs = sb.tile([128, TTB * NH], f32, tag="bs")
                bs3 = bs[:].rearrange("p (g n) -> p g n", g=TTB)
                # B banks are groups 1,3 (odd); A banks are 0,2
                nc.scalar.activation(bs3[:, :, :], ps3[:, 1::2, :], ACTF.Copy)
                nc.vector.tensor_tensor(pm3[:, sub * TTB:(sub + 1) * TTB, :],
                                        ps3[:, 0::2, :], bs3[:, :, :],
                                        op=ALU.min)
            nc.vector.tensor_reduce(mins[:, grp * RDB:(grp + 1) * RDB],
                                    pm3[:, :, :], axis=mybir.AxisListType.X,
                                    op=ALU.min)

        # ---------- final reduction ----------
        nc.vector.reduce_sum(cols[:, 0:1], mins[:], axis=mybir.AxisListType.X)
        nc.scalar.activation(cols[:, 0:1], cols[:, 0:1], ACTF.Copy,
                             scale=CHAMFER_SCALE)
        pf = psum.tile([1, 3], f32, tag="ps")
        nc.tensor.matmul(pf[:], ones_sb[:], cols[:], start=True, stop=True)
        out_sb = const.tile([1, 3], f32, tag="outsb")
        nc.scalar.activation(out_sb[:], pf[:], ACTF.Copy)
        nc.sync.dma_start(out_d[:], out_sb[:])

    nc.compile()
    return nc


def get_nc():
    if "nc" not in _NC_CACHE:
        _NC_CACHE["nc"] = _build_nc()
    return _NC_CACHE["nc"]


def shard_inputs(network_mesh, pc, fem_mesh):
    """Build the 8 per-core input maps (numpy slicing/layout only)."""
    network_mesh = np.ascontiguousarray(np.asarray(network_mesh, dtype=np.float32))
    pc = np.ascontiguousarray(np.asarray(pc, dtype=np.float32))
    fem_mesh = np.ascontiguousarray(np.asarray(fem_mesh, dtype=np.float32))
    ones_col = np.ones((128, 1), dtype=np.float32)
    ozrow = np.zeros((2, MSHARD), dtype=ml_dtypes.bfloat16)
    ozrow[0, :] = 1.0
    in_maps = []
    for k in range(8):
        b, h = k // 2, k % 2
        tops = np.ascontiguousarray(network_mesh[b, :, :, 15, :].reshape(3, N))
        pcs = pc[b, :, h * MSHARD:(h + 1) * MSHARD]
        pcsf = np.ascontiguousarray(pcs.reshape(96, 256))
        nmb = np.ascontiguousarray(
            network_mesh[b, :, h * 16:(h + 1) * 16, 0:15, :].reshape(128, 180))
        femb = np.ascontiguousarray(
            fem_mesh[b, :, h * 16:(h + 1) * 16, 0:15, :].reshape(128, 180))
        in_maps.append({
            "tops": tops, "pcsf": pcsf, "nmb": nmb, "femb": femb,
            "ones": ones_col, "ozrow": ozrow,
        })
    return in_maps


def kernel(network_mesh, pc, fem_mesh):
    from concourse.bass_utils import run_bass_kernel_spmd

    nc = get_nc()
    in_maps = shard_inputs(network_mesh, pc, fem_mesh)
    res = run_bass_kernel_spmd(nc, in_maps, list(range(8)))
    total = np.float64(0.0)
    for r in res.results:
        total += np.float64(np.sum(np.asarray(r["out"], dtype=np.float64)))
    return np.float32(total)


# revision 9
# speedup vs baseline: 2.1647x; 1.1423x over previous
"""Trainium2 Bass kernel for nn_MeshLoss (chamfer-to-top-surface + fem MSE).

Computation (see reference):
  top  = network_mesh[:, :, :, -1, :]    -> B x 1024 "top surface" points (3D)
  dist2[b, m] = min_n || pc[b,:,m] - top[b,:,n] ||^2
  out = mean(dist2) + mean((network_mesh[...,:15,:] - fem_mesh[...,:15,:])**2)

Distribution: 8 cores = (B=4) x (M-half=2). Each core computes a partial
scalar; host sums the 8x3 partials.

Per-core algorithm:
  dot(p~, t~_n) = ||p - t_n||^2 - ||p||^2 with p~ = [p;1], t~ = [-2t; ||t||^2].
  Matmuls run in bf16 hi/lo form stacked to K=16 (hh+hl+lh+ll accumulated in
  fp32 PSUM -> ~fp32-accurate dots, single-pass bf16-speed matmuls).
  Per 128-point tile: bankA = dots vs tops 0:512, bankB = vs tops 512:1024.
  ACT copies bankB to SBUF, DVE tensor_tensor-min(A, Bcopy) -> pm (batch-2
  tiles per op), then one 3D tensor_reduce-min per 8 tiles -> per-point mins.
  ||p||^2 and fem MSE are ACT square+accumulate passes. Final partition
  reduction is a ones-vector matmul; host adds the 3 partials per core.
"""

import numpy as np
import ml_dtypes
from contextlib import ExitStack

B = 4
M = 16384
MSHARD = M // 2          # 8192 points per core
N = 1024                 # top surface points per batch
NH = N // 2              # 512 = bank width
MT = MSHARD // 128       # 64 m-tiles per core
CHAMFER_SCALE = 1.0 / float(B * M)          # 1/65536
FEM_SCALE = 1.0 / float(B * 3 * 32 * 15 * 32)   # 1/184320
WEIGHT = 1.0
TTB = 2                  # m-tiles per TT-min op (PSUM batch)
RDB = 4                  # m-tiles per 3D-reduce op

_NC_CACHE = {}


def _build_nc():
    import concourse.bacc as bacc
    import concourse.tile as tile
    import concourse.mybir as mybir

    f32 = mybir.dt.float32
    bf16 = mybir.dt.bfloat16
    ACTF = mybir.ActivationFunctionType
    ALU = mybir.AluOpType

    nc = bacc.Bacc("TRN2", target_bir_lowering=False, debug=False, num_devices=8)

    tops_d = nc.dram_tensor("tops", [3, N], f32, kind="ExternalInput").ap()
    pcsf_d = nc.dram_tensor("pcsf", [96, 256], f32, kind="ExternalInput").ap()
    nmb_d = nc.dram_tensor("nmb", [128, 180], f32, kind="ExternalInput").ap()
    femb_d = nc.dram_tensor("femb", [128, 180], f32, kind="ExternalInput").ap()
    ones_d = nc.dram_tensor("ones", [128, 1], f32, kind="ExternalInput").ap()
    ozrow_d = nc.dram_tensor("ozrow", [2, MSHARD], bf16, kind="ExternalInput").ap()
    out_d = nc.dram_tensor("out", [1, 3], f32, kind="ExternalOutput").ap()

    with tile.TileContext(nc) as tc, ExitStack() as ctx:
        const = ctx.enter_context(tc.tile_pool(name="const", bufs=1))
        sb = ctx.enter_context(tc.tile_pool(name="sb", bufs=3))
        pmpool = ctx.enter_context(tc.tile_pool(name="pmp", bufs=2))
        psum = ctx.enter_context(tc.tile_pool(name="psum", bufs=2, space="PSUM"))

        # ---------- loads ----------
        top_sb = const.tile([3, N], f32, tag="top")
        nc.sync.dma_start(top_sb[:], tops_d[:])
        ones_sb = const.tile([128, 1], f32, tag="ones")
        nc.sync.dma_start(ones_sb[:], ones_d[:])
        pcsf_sb = const.tile([96, 256], f32, tag="pcsf")
        nc.sync.dma_start(pcsf_sb[:], pcsf_d[:])
        nmb_sb = const.tile([128, 180], f32, tag="nmb")
        nc.sync.dma_start(nmb_sb[:], nmb_d[:])
        femb_sb = const.tile([128, 180], f32, tag="femb")
        nc.sync.dma_start(femb_sb[:], femb_d[:])

        # ---------- prep: t4 = [-2t ; ||t||^2] (fp32) ----------
        sqt = const.tile([3, N], f32, tag="sqt")
        nc.scalar.activation(sqt[:], top_sb[:], ACTF.Square)
        pt = psum.tile([1, N], f32, tag="ps")
        nc.tensor.matmul(pt[:, 0:NH], ones_sb[0:3, :], sqt[:, 0:NH],
                         start=True, stop=True)
        nc.tensor.matmul(pt[:, NH:N], ones_sb[0:3, :], sqt[:, NH:N],
                         start=True, stop=True)
        t4 = const.tile([4, N], f32, tag="t4")
        nc.scalar.activation(t4[0:3, :], top_sb[:], ACTF.Copy, scale=-2.0)
        normsq = const.tile([1, N], f32, tag="normsq")
        nc.scalar.activation(normsq[:], pt[:], ACTF.Copy)
        nc.sync.dma_start(t4[3:4, :], normsq[:])

        # ---------- bf16 hi/lo decomposition ----------
        # rhs16 = [t4_hi; t4_lo; t4_hi; t4_lo]  (K=16)
        th = const.tile([4, N], bf16, tag="th")
        nc.vector.tensor_copy(th[:], t4[:])
        tl = const.tile([4, N], bf16, tag="tl")
        nc.vector.tensor_sub(tl[:], t4[:], th[:])
        t16 = const.tile([16, N], bf16, tag="t16")
        nc.sync.dma_start(t16[0:4, :], th[:])
        nc.sync.dma_start(t16[4:8, :], tl[:])
        nc.sync.dma_start(t16[8:12, :], th[:])
        nc.sync.dma_start(t16[12:16, :], tl[:])

        # lhsT16 = [p_hi;1; p_hi;1; p_lo;0; p_lo;0]  (K=16, M=8192)
        # p hi/lo computed on the [128,192] layout, DMA'd into the [3,8192]
        # rows (identical flat element order).
        ph = const.tile([96, 256], bf16, tag="ph")
        nc.vector.tensor_copy(ph[:], pcsf_sb[:])
        pl = const.tile([96, 256], bf16, tag="pl")
        nc.vector.tensor_sub(pl[:], pcsf_sb[:], ph[:])
        p16 = const.tile([16, MSHARD], bf16, tag="p16")
        nc.sync.dma_start(p16[0:3, :], ph[:])
        nc.sync.dma_start(p16[3:4, :], ozrow_d[0:1, :])
        nc.sync.dma_start(p16[4:7, :], ph[:])
        nc.sync.dma_start(p16[7:8, :], ozrow_d[0:1, :])
        nc.sync.dma_start(p16[8:11, :], pl[:])
        nc.sync.dma_start(p16[11:12, :], ozrow_d[1:2, :])
        nc.sync.dma_start(p16[12:15, :], pl[:])
        nc.sync.dma_start(p16[15:16, :], ozrow_d[1:2, :])

        mins = const.tile([128, MT], f32, tag="mins")
        cols = const.tile([128, 3], f32, tag="cols")
        nc.vector.memset(cols[:], 0.0)

        # ---------- ||p||^2 and fem MSE partials ----------
        p2j = pmpool.tile([96, 256], f32, tag="p2j")
        nc.scalar.activation(p2j[:], pcsf_sb[:], ACTF.Square,
                             scale=float(np.sqrt(CHAMFER_SCALE)),
                             accum_out=cols[0:96, 1:2])
        fdiff = pmpool.tile([128, 180], f32, tag="fdiff")
        nc.vector.tensor_sub(fdiff[:], nmb_sb[:], femb_sb[:])
        fj = pmpool.tile([128, 180], f32, tag="fj")
        nc.scalar.activation(fj[:], fdiff[:], ACTF.Square,
                             scale=float(np.sqrt(FEM_SCALE * WEIGHT)),
                             accum_out=cols[:, 2:3])

        # ---------- main chamfer loop ----------
        # PSUM slot [128, 2048] = [A1|B1|A2|B2] for 2 m-tiles.
        assert MT % RDB == 0 and RDB % TTB == 0
        for grp in range(MT // RDB):       # 8 groups of 8 m-tiles
            pmbig = pmpool.tile([128, RDB * NH], f32, tag="pmbig")
            pm3 = pmbig[:].rearrange("p (g n) -> p g n", g=RDB)
            for sub in range(RDB // TTB):  # 4 PSUM slots of 2 m-tiles
                ps = psum.tile([128, TTB * N], f32, tag="ps")
                for j in range(TTB):
                    mt = grp * RDB + sub * TTB + j
                    lhsT = p16[:, mt * 128:(mt + 1) * 128]
                    nc.tensor.matmul(ps[:, j * N:j * N + NH],
                                     lhsT, t16[:, 0:NH], start=True, stop=True)
                    nc.tensor.matmul(ps[:, j * N + NH:(j + 1) * N],
                                     lhsT, t16[:, NH:N], start=True, stop=True)
                ps3 = ps[:].rearrange("p (g n) -> p g n", g=2 * TTB)
                bs = sb.tile([128, TTB * NH], f32, tag="bs")
                bs3 = bs[:].rearrange("p (g n) -> p g n", g=TTB)
                # B banks are groups 1,3 (odd); A banks are 0,2
                nc.scalar.activation(bs3[:, :, :], ps3[:, 1::2, :], ACTF.Copy)
                nc.vector.tensor_tensor(pm3[:, sub * TTB:(sub + 1) * TTB, :],
                                        ps3[:, 0::2, :], bs3[:, :, :],
                                        op=ALU.min)
            nc.vector.tensor_reduce(mins[:, grp * RDB:(grp + 1) * RDB],
                                    pm3[:, :, :], axis=mybir.AxisListType.X,
                                    op=ALU.min)

        # ---------- final reduction ----------
        nc.vector.reduce_sum(cols[:, 0:1], mins[:], axis=mybir.AxisListType.X)
        nc.scalar.activation(cols[:, 0:1], cols[:, 0:1], ACTF.Copy,
                             scale=CHAMFER_SCALE)
        pf = psum.tile([1, 3], f32, tag="ps")
        nc.tensor.matmul(pf[:], ones_sb[:], cols[:], start=True, stop=True)
        out_sb = const.tile([1, 3], f32, tag="outsb")
        nc.scalar.activation(out_sb[:], pf[:], ACTF.Copy)
        nc.sync.dma_start(out_d[:], out_sb[:])

    nc.compile()
    return nc


def get_nc():
    if "nc" not in _NC_CACHE:
        _NC_CACHE["nc"] = _build_nc()
    return _NC_CACHE["nc"]


def shard_inputs(network_mesh, pc, fem_mesh):
    """Build the 8 per-core input maps (numpy slicing/layout only)."""
    network_mesh = np.ascontiguousarray(np.asarray(network_mesh, dtype=np.float32))
    pc = np.ascontiguousarray(np.asarray(pc, dtype=np.float32))
    fem_mesh = np.ascontiguousarray(np.asarray(fem_mesh, dtype=np.float32))
    ones_col = np.ones((128, 1), dtype=np.float32)
    ozrow = np.zeros((2, MSHARD), dtype=ml_dtypes.bfloat16)
    ozrow[0, :] = 1.0
    in_maps = []
    for k in range(8):
        b, h = k // 2, k % 2
        tops = np.ascontiguousarray(network_mesh[b, :, :, 15, :].reshape(3, N))
        pcs = pc[b, :, h * MSHARD:(h + 1) * MSHARD]
        pcsf = np.ascontiguousarray(pcs.reshape(96, 256))
        nmb = np.ascontiguousarray(
            network_mesh[b, :, h * 16:(h + 1) * 16, 0:15, :].reshape(128, 180))
        femb = np.ascontiguousarray(
            fem_mesh[b, :, h * 16:(h + 1) * 16, 0:15, :].reshape(128, 180))
        in_maps.append({
            "tops": tops, "pcsf": pcsf, "nmb": nmb, "femb": femb,
            "ones": ones_col, "ozrow": ozrow,
        })
    return in_maps


def kernel(network_mesh, pc, fem_mesh):
    from concourse.bass_utils import run_bass_kernel_spmd

    nc = get_nc()
    in_maps = shard_inputs(network_mesh, pc, fem_mesh)
    res = run_bass_kernel_spmd(nc, in_maps, list(range(8)))
    total = np.float64(0.0)
    for r in res.results:
        total += np.float64(np.sum(np.asarray(r["out"], dtype=np.float64)))
    return np.float32(total)
